# revision 23
# baseline (speedup 1.0000x reference)
"""B3-spline undecimated wavelet transform (a-trous, 3 levels) on 8 trn2 cores.

kernel(x: [16, 1024, 1024] f32) -> [16, 4, 1024, 1024] f32  ([w1, w2, w3, c3])

Sharding: pure data parallel, batch 16 -> 2 images per NeuronCore.

Per-core kernel: each level's separable dilated 5x5 B3 smoothing is fused
into 5 PSUM-accumulated banded matmuls on the tensor engine:
    y'[h, w] = sum_k W5[k] * (A_d @ y)[h, w + (k-2)*d]
A_d is the banded H-conv matrix with reflect padding folded into top/bottom
blocks; the W-shift is a free-axis offset on the rhs AP; W-reflect comes from
8 mirrored pad columns in SBUF. H uses overlapping 128-row tiles (stride 112).

Scheduling (final, measured 172us HW vs 206-216us baseline):
- Input loads are per-tile (9 DMAs/image) so compute starts ~2.5us in
  instead of after a 33us monolithic load. The raw image lives in its
  own 2-buffer pool, separate from the level-output pool, so image
  n+1's loads prefetch during image n's levels 1-2.
- The 27 scaled conv matrices ship as one inline [128, 27*128] tensor
  in 3 DMAs (level-0 top-class mats first, so tile 0 gates on 200KB).
- Halo seams: SBUF->SBUF DMA measures only ~25GB/s on HW (4.3us per
  4-block group), so levels 0/1 compute their neighbors' halo rows
  in the SAME matmuls via extended banded matrices (VALID ranges; free,
  since PE cost is per-column): l0 emits rows [2,126) making l0->l1
  fully seam-free, l1 emits [6,122) so l1->l2 needs only 6-row seams
  (4 batched DMAs per transition, issued at t=4/t=8). Compute engines
  cannot do these copies (BIR verifier rejects non-32-aligned
  partition bases).
- Outputs flush in 2-tile row-chunks per channel as tiles complete
  (after t=1,3,5,7,8), split across gpsimd-SWDGE (w1,w3) and ACT-HWDGE
  (w2,c3) queues (measured ~256 / ~298 GB/s per queue); w staging is a
  rolling 4-block buffer (flushes trail writes by two tiles); c3 is
  evacuated into a yn-pool tile (PW layout) and flushed from there.
Ablations (HW): compute-only 130us, +input/seams 136us, full 166-172us;
HBM mandatory traffic floor ~120us, PE stream ~112us (208ns/matmul at
full clock, measured). fp32r matmul runs at 1 col/cycle @2.4GHz; a
palindromic tap reorder + seams-on-ACT variant measured SLOWER (179us),
so the straightforward per-half 5-tap groups are kept.
"""
import sys
sys.path.insert(0, "/opt/trn_rl_repo")
import contextlib
import numpy as np
import concourse.bass as bass
import concourse.mybir as mybir
from concourse import bacc
from concourse.tile import TileContext

DT = mybir.dt
F32 = DT.float32
F32R = DT.float32r

H = W = 1024
PAD = 8
PW = W + 2 * PAD
NT = 9
STRIDE = 112
DILS = (1, 2, 4)
W5 = np.array([1.0, 4.0, 6.0, 4.0, 1.0]) / 16.0
TAP_ORDER = (0, 4, 1, 3, 2)
SCALE_OF_TAP = {0: 0, 4: 0, 1: 1, 3: 1, 2: 2}
SCALES = (1.0 / 16.0, 4.0 / 16.0, 6.0 / 16.0)
CLS_ORDER = ("top", "int", "bot")


def tile_geom(t):
    if t == 0:
        return 0, 120, 0
    if t == NT - 1:
        return STRIDE * t + 8, 120, 8
    return STRIDE * t + 8, 112, 8


# Valid output partition range [lo, hi) per (level, tile class). Levels 0/1
# produce extra halo rows (free: matmul cost is per-column) so the next
# level's K-window reads need no (l0->l1) or only 6-row (l1->l2) seams:
#   l1 cols [6,122) read yn0 rows [2,126) = l0's extended validity;
#   l2 cols [8,120) read yn1 rows [0,128) = l1's [6,122) + 6-row seams.
VALID = {
    0: {"top": (0, 126), "int": (2, 126), "bot": (2, 128)},
    1: {"top": (0, 122), "int": (6, 122), "bot": (6, 128)},
    2: {"top": (0, 120), "int": (8, 120), "bot": (8, 128)},
}


def build_A(cls, d, lo, hi):
    A = np.zeros((128, 128), np.float64)
    for col in range(lo, hi):
        for i in range(5):
            if cls == "int":
                k = col + (i - 2) * d
            elif cls == "top":
                g = col + (i - 2) * d
                k = -g if g < 0 else g
            else:
                g = 896 + col + (i - 2) * d
                k = (2046 - g if g > 1023 else g) - 896
            A[k, col] += W5[i]
    return A


def build(n_img=2, reps=1, bench=False, n_cores=8, ablate=()):
    nc = bacc.Bacc(trn_type="TRN2", target_bir_lowering=False, debug=False,
                   num_devices=n_cores)
    x_d = nc.dram_tensor("x", [n_img, H, W], F32R, kind="ExternalInput")
    if bench:
        o_d = nc.dram_tensor("o", [n_img, 4, H, W], F32, kind="Internal")
        bench_d = nc.dram_tensor("bench_out", [1, 1], F32,
                                 kind="ExternalOutput")
    else:
        o_d = nc.dram_tensor("o", [n_img, 4, H, W], F32,
                             kind="ExternalOutput")
        bench_d = None

    # all 27 scaled matrices side by side: idx = li*9 + cls*3 + si
    mats_np = np.concatenate(
        [(build_A(cls, d, *VALID[li][cls]) * s).astype(np.float32)
         for li, d in enumerate(DILS)
         for cls in CLS_ORDER
         for s in SCALES], axis=1)
    mats_d = nc.inline_tensor(mats_np, name="mats")

    with TileContext(nc) as tc:
        ctx = contextlib.ExitStack()
        with ctx:
            consts = ctx.enter_context(tc.tile_pool(name="consts", bufs=1))
            xpool = ctx.enter_context(tc.tile_pool(name="xbuf", bufs=2))
            ypool = ctx.enter_context(tc.tile_pool(name="ybuf", bufs=2))
            psum = ctx.enter_context(tc.tile_pool(name="acc", bufs=8,
                                                  space="PSUM"))
            wstage = ctx.enter_context(tc.tile_pool(name="wstage", bufs=2))

            mats = consts.tile([128, 27 * 128], F32R, tag="mats", name="mats")
            nc.scalar.dma_start(out=mats[:, 0:3 * 128],
                                in_=mats_d[:, 0:3 * 128].bitcast(F32R))
            nc.scalar.dma_start(out=mats[:, 3 * 128:9 * 128],
                                in_=mats_d[:, 3 * 128:9 * 128].bitcast(F32R))
            nc.scalar.dma_start(out=mats[:, 9 * 128:27 * 128],
                                in_=mats_d[:, 9 * 128:27 * 128].bitcast(F32R))

            def mslice(li, cls, si):
                idx = li * 9 + CLS_ORDER.index(cls) * 3 + si
                return mats[:, idx * 128:(idx + 1) * 128]

            def fill_pads(ybig, t):
                b = PW * t
                nc.vector.tensor_copy(ybig[:, b:b + PAD],
                                      ybig[:, b + 2 * PAD:b + PAD:-1])
                nc.vector.tensor_copy(ybig[:, b + W + PAD:b + W + 2 * PAD],
                                      ybig[:, b + W + PAD - 2:b + W - 2:-1])

            def seam_group(yn, g):
                # 6-row full-width (pads included) halo copies, 4 blocks per
                # DMA (compute engines reject non-32-aligned partition bases,
                # so these must be DMAs). Only l1->l2 needs them (see VALID).
                # On the gpsimd SWDGE queue: SP then carries only input
                # loads, so image n+1's loads are not stuck behind seam
                # waits and run during level 1 (HBM slack) instead of
                # colliding with the level-2 flush burst. SWDGE is idle
                # during level 1 (w1 flushed in l0, w3 in l2), so the slow
                # ~4.3us SBUF->SBUF seam transfers slot into its gap.
                if g == 0:
                    nc.gpsimd.dma_start(out=yn[0:6, PW:PW * 5],
                                        in_=yn[112:118, 0:PW * 4])
                    nc.gpsimd.dma_start(out=yn[122:128, 0:PW * 4],
                                        in_=yn[10:16, PW:PW * 5])
                else:
                    nc.gpsimd.dma_start(out=yn[0:6, PW * 5:PW * 9],
                                        in_=yn[112:118, PW * 4:PW * 8])
                    nc.gpsimd.dma_start(out=yn[122:128, PW * 4:PW * 8],
                                        in_=yn[10:16, PW * 5:PW * 9])

            def flush_w(big, img, ch, t, eng):
                # rolling 4-block staging: block t lives at col (t%4)*1024
                if t == 1:
                    eng.dma_start(out=o_d[img, ch, 0:120, :],
                                  in_=big[0:120, 0:1024])
                    eng.dma_start(out=o_d[img, ch, 120:232, :],
                                  in_=big[8:120, 1024:2048])
                elif t == NT - 1:
                    eng.dma_start(out=o_d[img, ch, 904:1024, :],
                                  in_=big[8:128, 0:1024])
                else:
                    r0 = 112 * (t - 1) + 8
                    c0 = ((t - 1) % 4) * 1024
                    eng.dma_start(
                        out=o_d[img, ch, r0:r0 + 224, :].rearrange(
                            "(t p) w -> p t w", t=2),
                        in_=big[8:120, c0:c0 + 2048].rearrange(
                            "p (t w) -> p t w", t=2))

            def flush_c3(yn, img, t):
                eng = nc.scalar
                if t == 1:
                    eng.dma_start(out=o_d[img, 3, 0:120, :],
                                  in_=yn[0:120, PAD:PAD + W].bitcast(F32))
                    eng.dma_start(out=o_d[img, 3, 120:232, :],
                                  in_=yn[8:120, PW + PAD:PW + PAD + W]
                                  .bitcast(F32))
                elif t == NT - 1:
                    eng.dma_start(
                        out=o_d[img, 3, 904:1024, :],
                        in_=yn[8:128, 8 * PW + PAD:8 * PW + PAD + W]
                        .bitcast(F32))
                else:
                    r0 = 112 * (t - 1) + 8
                    eng.dma_start(
                        out=o_d[img, 3, r0:r0 + 224, :].rearrange(
                            "(t p) w -> p t w", t=2),
                        in_=yn[8:120, (t - 1) * PW:(t + 1) * PW].rearrange(
                            "p (t w) -> p t w", t=2)[:, :, PAD:PAD + W]
                        .bitcast(F32))

            def body(img):
                ybig = xpool.tile([128, NT * PW], F32R, tag="ybig",
                                  name="ybig")
                for t in range(NT):
                    if "input" not in ablate:
                        nc.sync.dma_start(
                            out=ybig[:, PW * t + PAD:PW * t + PAD + W],
                            in_=bass.AP(x_d, (img * H + t * STRIDE) * W,
                                        [[W, 128], [1, W]]))
                    fill_pads(ybig, t)
                ycur = ybig

                for li, d in enumerate(DILS):
                    last = (li == len(DILS) - 1)
                    ynext = ypool.tile([128, NT * PW], F32R, tag="ynext",
                                       name=("c3big" if last else "ynbig"))
                    wbig = wstage.tile([128, 4 * 1024], F32, tag="wbig",
                                       name="wbig")
                    w_eng = nc.scalar if li == 1 else nc.gpsimd
                    for t in range(NT):
                        cls = ("top" if t == 0
                               else ("bot" if t == NT - 1 else "int"))
                        wcol = (t % 4) * 1024
                        for c in range(2):
                            col = PAD + 512 * c
                            acc = psum.tile([128, 512], F32, tag="acc",
                                            name="acc")
                            for j, i in enumerate(TAP_ORDER):
                                sh = PW * t + col + (i - 2) * d
                                nc.tensor.matmul(
                                    acc[:],
                                    mslice(li, cls, SCALE_OF_TAP[i]),
                                    ycur[:, sh:sh + 512],
                                    start=(j == 0), stop=(j == 4))
                            nc.scalar.copy(
                                ynext[:, PW * t + col:PW * t + col + 512],
                                acc[:])
                            wslice = wbig[:, wcol + 512 * c:
                                          wcol + 512 * c + 512]
                            y0s = ycur[:, PW * t + col:
                                       PW * t + col + 512].bitcast(F32)
                            if (2 * t + c) % 2 == 0:
                                nc.vector.tensor_tensor(
                                    wslice, y0s, acc[:],
                                    mybir.AluOpType.subtract)
                            else:
                                y1s = ynext[:, PW * t + col:
                                            PW * t + col + 512].bitcast(F32)
                                nc.gpsimd.tensor_tensor(
                                    wslice, y0s, y1s,
                                    mybir.AluOpType.subtract)
                        if not last:
                            fill_pads(ynext, t)
                            if li == 1 and "seam" not in ablate:
                                if t == 4:
                                    seam_group(ynext, 0)
                                elif t == NT - 1:
                                    seam_group(ynext, 1)
                        if t in (1, 3, 5, 7, NT - 1) and "flush" not in ablate:
                            flush_w(wbig, img, li, t, w_eng)
                            if last:
                                flush_c3(ynext, img, t)
                    ycur = ynext

            if bench and reps > 1:
                with tc.For_i(0, reps, name="bench_reps"):
                    for img in range(n_img):
                        body(img)
            else:
                for img in range(n_img):
                    body(img)
            if bench:
                done = consts.tile([1, 1], F32, tag="done", name="done")
                nc.vector.memset(done[:], 1.0)
                nc.sync.dma_start(out=bench_d[0:1, 0:1], in_=done[:])

    nc.compile()
    return nc


_NC = None


def kernel(x):
    global _NC
    x = np.ascontiguousarray(np.asarray(x), dtype=np.float32)
    B = x.shape[0]
    n_cores = 8
    per = B // n_cores
    if _NC is None:
        _NC = build(n_img=per, n_cores=n_cores)
    from concourse.bass_utils import run_bass_kernel_spmd
    ins = [{"x": np.ascontiguousarray(x[per * c:per * c + per])}
           for c in range(n_cores)]
    res = run_bass_kernel_spmd(_NC, ins, core_ids=list(range(n_cores)))
    return np.concatenate([r["o"] for r in res.results], axis=0)
